# revision 22
# baseline (speedup 1.0000x reference)
"""Trainium2 Bass kernel for nn_KANLayer (piecewise-constant KAN forward).

Math: out[b,t,i] = sum_j D[i,j,seg(x_tj)] with D[i,j,s] = c_s+c_{s+1}+c_{s+2},
seg = which of the 9 knot intervals x falls in. Telescoping over s with step
planes step_s[t,j] = [seg >= s] gives a K=512*8=4096 matmul with a 0/1 left
operand plus a free per-i constant (base).

Full-fp8 design (vs the 127.9us bf16 baseline, PE-bound at 109us):
  * All 4096 K-lanes run in fp8e4m3 with DoubleRow perf mode (2 K-lanes per
    PE cell per cycle; measured on HW: DR matmuls pace at the same 216ns as
    bf16 N=512). PE work: 256 MMs x 216ns ~= 55us.
  * The G_s increments are quantized by a per-lane beam-search DP minimizing
    the seg-uniform expected squared error of the cumulative sums, with the
    per-lane mean error folded into the exact-f32 base (mean-centering).
    Measured end-to-end (deterministic fixed-seed inputs): rel err 1.6e-2.
  * g-stationary orientation: out[i-chunk, token]; LDWEIGHTS (135ns) hides
    under the 216ns matmuls.
  * Step-plane builds are split across two engines so neither is critical:
    chunks 0-7 (j 0..255) as 0/1 is_ge planes on VectorE, chunks 8-15
    (j 256..511) as +-1 sign planes on ScalarE with halved coefficients
    (exact exponent shift -> identical precision). Build FIFOs are ordered
    so evacuations never sit behind not-yet-consumed builds.
  * PSUM evacuation adds base and downcasts to bf16 (half the out DMA and
    an accelerated DVE/ACT mode); the host upcasts. Out DMAs ride the
    otherwise-idle sync ring.

Sharding: data-parallel, 2048 tokens per core; g/base replicated. Output is
produced transposed ([i, token]) and untransposed on the host.
"""

import hashlib
from contextlib import ExitStack

import numpy as np
import ml_dtypes

import concourse.bass as bass  # noqa: F401
import concourse.tile as tile
from concourse import bacc, mybir
from concourse.bass_utils import run_bass_kernel_spmd

N_CORES = 8
TOK = 2048          # tokens per core
IN_F = 512
OUT_F = 512
GTOK = 512          # tokens per group
N_GRP = TOK // GTOK  # 4
IC = 4              # i-feature chunks of 128
NC8 = 16            # fp8 DoubleRow chunks: (j-block of 128) x (s-pair), 4x4
# chunks built on ScalarE as sign (+-1) planes with halved coefficients; the
# rest are is_ge (0/1) planes on VectorE. Alternating over the early chunks
# makes the combined build pace (~1.1us/chunk) beat the PE's consumption
# (1.73us/chunk) from a standing start; vector ~455ns/op vs scalar ~700ns/op
# puts 10 chunks on vector and 6 on scalar.
SIGN_CHUNKS = (1, 3, 5, 7, 9, 11)
BF16 = mybir.dt.bfloat16
F8 = mybir.dt.float8e4
F32 = mybir.dt.float32
E4NP = ml_dtypes.float8_e4m3

_PROGRAM_CACHE = {}
_WEIGHT_CACHE = {}


def _build_program():
    nc = bacc.Bacc("TRN2", target_bir_lowering=False, debug=False)

    segb_d = nc.dram_tensor("segb", [128, 4, TOK], BF16, kind="ExternalInput").ap()
    g8_d = nc.dram_tensor("g8", [128, NC8, 2, IC, 128], F8, kind="ExternalInput").ap()
    base_d = nc.dram_tensor("base", [128, IC], F32, kind="ExternalInput").ap()
    # out[ic, p, tok] -> feature i = ic*128 + p (bf16; host upcasts)
    out_d = nc.dram_tensor("out", [IC, 128, TOK], BF16, kind="ExternalOutput").ap()

    with tile.TileContext(nc) as tc, ExitStack() as ctx:
        seg_pool = ctx.enter_context(tc.tile_pool(name="seg", bufs=1))
        g8_pool = ctx.enter_context(tc.tile_pool(name="g8", bufs=1))
        base_pool = ctx.enter_context(tc.tile_pool(name="base", bufs=1))
        wm_pool = ctx.enter_context(tc.tile_pool(name="wm", bufs=1))
        st8_pool = ctx.enter_context(tc.tile_pool(name="st8", bufs=1))
        out_pool = ctx.enter_context(tc.tile_pool(name="out", bufs=4))
        psum_pool = ctx.enter_context(tc.tile_pool(name="psum", bufs=1, space="PSUM"))

        # --- PE warmup: un-throttle the HAM clock gate before real work
        # (covers the ~3.4us activity window plus the input-DMA latency).
        wm = wm_pool.tile([128, 384], BF16, name="wm")
        nc.vector.memset(wm[:], 0.0)
        ps_w = psum_pool.tile([128, 2 * GTOK], F32, name="ps_0")
        for _ in range(22):
            nc.tensor.matmul(
                ps_w[:, :256], wm[:, :128], wm[:, 128:384],
                start=True, stop=True, skip_group_check=True,
            )

        # --- input DMAs.  seg pieces on the sync HWDGE ring (group-major so
        # group 0 lands first), g chunks on the scalar ring in consumption
        # order (first 4 singly for a fast start, the rest batched to keep
        # the scalar engine's issue cost low), base on gpsimd.
        # seg pieces jb-major per super-half: the matmul stream consumes
        # chunks jb-major, and each piece covers BOTH groups of a super, so
        # piece k is needed ~7us after piece k-1 while they land ~1.4us
        # apart. (Group-major 16-piece order starved the early builds: the
        # piece for group 1 / jb0 -- needed by the second matmul -- was 5th.)
        segb_t = seg_pool.tile([128, 4, TOK], BF16, name="segb")
        for half in range(2):
            sl = slice(half * 2 * GTOK, (half + 1) * 2 * GTOK)
            for jb in range(4):
                nc.sync.dma_start(segb_t[:, jb, sl], segb_d[:, jb, sl])

        g8_t = g8_pool.tile([128, NC8, 2, IC, 128], F8, name="g8")
        for c0, c1 in ((0, 4), (4, 16)):
            nc.scalar.dma_start(g8_t[:, c0:c1], g8_d[:, c0:c1])

        base_t = base_pool.tile([128, IC], F32, name="base")
        nc.gpsimd.dma_start(base_t[:], base_d[:])

        # per-level sign biases (0.5 - s) as [128,1] columns for ACT sign
        bias_t = base_pool.tile([128, 8], F32, name="sgnbias")
        for s in range(1, 9):
            nc.gpsimd.memset(bias_t[:, s - 1 : s], 0.5 - float(s))

        # --- step-plane builds. chunk c = jb*4 + sp covers lanes
        # (j = jb*128 + p, s = 2*sp + 1 + b). VectorE owns chunks 0..7
        # (is_ge -> 0/1), ScalarE owns chunks 8..15 (sign -> -1/+1, with
        # halved g so the net step coefficient is unchanged). Interleaved by
        # chunk across the two groups of each super so the PE never waits.
        st8 = [st8_pool.tile([128, NC8, 2, GTOK], F8, name=f"st8_{q}") for q in range(N_GRP)]

        def emit_build(c, q):
            jb, sp = c // 4, c % 4
            sl = slice(q * GTOK, (q + 1) * GTOK)
            for b in range(2):
                s = 2 * sp + 1 + b
                if c in SIGN_CHUNKS:
                    nc.scalar.sign(
                        st8[q][:, c, b, :], segb_t[:, jb, sl],
                        bias=bias_t[:, s - 1 : s],
                    )
                else:
                    nc.vector.tensor_scalar(
                        st8[q][:, c, b, :], segb_t[:, jb, sl],
                        float(s) - 0.5, None, mybir.AluOpType.is_ge,
                    )

        def emit_mms(sup, ps):
            groups = (2 * sup, 2 * sup + 1)
            for c in range(NC8):
                for ic in range(IC):
                    for qi, q in enumerate(groups):
                        nc.tensor.matmul(
                            ps[ic][:, qi * GTOK : (qi + 1) * GTOK],
                            g8_t[:, c, :, ic, :],
                            st8[q][:, c, :, :],
                            start=c == 0, stop=c == NC8 - 1,
                            perf_mode=mybir.MatmulPerfMode.DoubleRow,
                        )

        def emit_evac(sup, ps, ic, on_vector, ndma):
            # one [128, 1024] op evacuates the whole i-chunk (both banks),
            # adds base, downcasts to bf16; one contiguous DMA ships it
            groups = (2 * sup, 2 * sup + 1)
            ot = out_pool.tile([128, 2 * GTOK], BF16, name="ot")
            if on_vector:
                nc.vector.tensor_scalar(
                    ot[:], ps[ic][:], base_t[:, ic : ic + 1],
                    None, mybir.AluOpType.add,
                )
            else:
                nc.scalar.add(ot[:], ps[ic][:], base_t[:, ic : ic + 1])
            eng = nc.sync if ndma % 2 == 0 else nc.gpsimd
            eng.dma_start(
                out_d[ic][:, groups[0] * GTOK : (groups[0] + 2) * GTOK], ot[:]
            )

        # Emission order sets each engine's FIFO. Builds are emitted as
        # early as possible; each engine slots its share of the super-0
        # evacuations (gated by super-0's stop matmuls) between super-1
        # build segments whose deadlines leave room, so neither delays the
        # other. Evacuations go ic-interleaved V/S in bank-reuse order.
        ps0 = [psum_pool.tile([128, 2 * GTOK], F32, name=f"ps_{ic}")
               for ic in range(IC)]
        for c in range(NC8):
            for q in (0, 1):
                emit_build(c, q)
        # super-1 builds, early part: vector chunks 0-4, scalar chunks 1,3
        for c in (0, 1, 2, 3, 4):
            for q in (2, 3):
                emit_build(c, q)
        emit_mms(0, ps0)
        emit_evac(0, ps0, 0, False, 0)
        emit_evac(0, ps0, 1, True, 1)
        emit_evac(0, ps0, 2, False, 0)
        emit_evac(0, ps0, 3, True, 1)
        ps1 = [psum_pool.tile([128, 2 * GTOK], F32, name=f"ps_{ic}")
               for ic in range(IC)]
        # super-1 builds, late part
        for c in (5, 6, 7, 8, 9, 10, 11, 12, 13, 14, 15):
            for q in (2, 3):
                emit_build(c, q)
        emit_mms(1, ps1)
        emit_evac(1, ps1, 0, False, 0)
        emit_evac(1, ps1, 1, True, 1)
        emit_evac(1, ps1, 2, False, 0)
        emit_evac(1, ps1, 3, True, 1)

    nc.compile()
    return nc


def _get_program():
    if "nc" not in _PROGRAM_CACHE:
        _PROGRAM_CACHE["nc"] = _build_program()
    return _PROGRAM_CACHE["nc"]


# sorted finite fp8e4m3 grid for the DP quantizer
_E4_GRID = np.arange(256, dtype=np.uint8).view(E4NP).astype(np.float32)
_E4_GRID = np.unique(_E4_GRID[np.isfinite(_E4_GRID)])


def _dp_quant(Gsub: np.ndarray, B: int = 8) -> np.ndarray:
    """Beam-DP quantization of cumulative increments onto the fp8e4m3 grid.

    Minimizes sum_s e_s^2 - (sum_s e_s)^2/9 per lane, where e_s is the
    partial-sum error at level s (seg uniform over 9 values; the mean term
    is folded into base by the caller)."""
    grid, NG = _E4_GRID, len(_E4_GRID)
    S, L = Gsub.shape
    P = np.cumsum(Gsub, axis=0)
    Pq = np.zeros((1, L), np.float32)
    se = np.zeros((1, L), np.float32)
    s2 = np.zeros((1, L), np.float32)
    paths = np.zeros((1, S, L), np.float32)
    for s in range(S):
        tgt = P[s][None, :] - Pq
        idx = np.searchsorted(grid, tgt.ravel()).reshape(tgt.shape)
        B0 = Pq.shape[0]
        offs = (-1, 0)
        cand = [grid[np.clip(idx + o, 0, NG - 1)] for o in offs]
        newPq = np.concatenate([Pq + qc for qc in cand], axis=0)
        e = P[s][None, :] - newPq
        newse = np.tile(se, (len(offs), 1)) + e
        news2 = np.tile(s2, (len(offs), 1)) + e * e
        newpaths = np.tile(paths, (len(offs), 1, 1))
        for k, qc in enumerate(cand):
            newpaths[k * B0 : (k + 1) * B0, s, :] = qc
        rem = S - 1 - s
        pse = newse + e * rem
        ps2 = news2 + e * e * rem
        cost = ps2 - pse * pse / 9.0
        B_eff = min(B, cost.shape[0])
        sel = np.argpartition(cost, B_eff - 1, axis=0)[:B_eff]
        Pq = np.take_along_axis(newPq, sel, 0)
        se = np.take_along_axis(newse, sel, 0)
        s2 = np.take_along_axis(news2, sel, 0)
        paths = np.take_along_axis(newpaths, sel[:, None, :], 0)
    best = np.argmin(s2 - se * se / 9.0, axis=0)
    return np.take_along_axis(paths, best[None, None, :], 0)[0]


def _prep_weights(coeffs: np.ndarray):
    key = hashlib.sha1(np.ascontiguousarray(coeffs).tobytes()).hexdigest()
    if key in _WEIGHT_CACHE:
        return _WEIGHT_CACHE[key]
    c = coeffs.astype(np.float32)
    # G[s-1][j, i] = c[i,j,s+2] - c[i,j,s-1]; base[i] = sum_j (c0+c1+c2)
    G = np.empty((8, IN_F, OUT_F), dtype=np.float32)
    for s in range(1, 9):
        G[s - 1] = (c[:, :, s + 2] - c[:, :, s - 1]).T
    base = (c[:, :, 0] + c[:, :, 1] + c[:, :, 2]).sum(axis=1).astype(np.float32)

    Gq = _dp_quant(G.reshape(8, -1)).reshape(8, IN_F, OUT_F)
    # sign-plane chunks (SIGN_CHUNKS, built as +-1 on ScalarE): store h = Gq/2
    # (the fp8 grid is exponent-self-similar, so this is exact except at the
    # subnormal floor). sum_s h*sgn = sum_s (2h)*step - sum_s h, so the
    # effective step coefficient is 2h and the constant folds into base.
    G_store = Gq.copy()
    Gq_eff = Gq.copy()
    hsum = np.zeros(OUT_F, dtype=np.float32)
    for c8 in SIGN_CHUNKS:
        jb, sp = c8 // 4, c8 % 4
        jsl = slice(jb * 128, (jb + 1) * 128)
        for s_idx in (2 * sp, 2 * sp + 1):
            h = (Gq[s_idx, jsl, :] * 0.5).astype(E4NP).astype(np.float32)
            G_store[s_idx, jsl, :] = h
            Gq_eff[s_idx, jsl, :] = 2.0 * h
            hsum += h.sum(axis=0)
    # mean-centering with the effective coefficients + sign-plane constant
    E = np.cumsum(G, axis=0) - np.cumsum(Gq_eff, axis=0)
    base_adj = base + (E.sum(axis=0) / 9.0).sum(axis=0) + hsum

    # g8[p, c8 = jb*4+sp, b, ic, m] = G_store[2*sp+b, jb*128+p, ic*128+m]
    Gf = G_store.reshape(4, 2, 4, 128, IC, 128)  # [sp, b, jb, p, ic, m]
    g8 = np.ascontiguousarray(
        Gf.transpose(3, 2, 0, 1, 4, 5).reshape(128, NC8, 2, IC, 128)
    ).astype(E4NP)
    base_tile = np.ascontiguousarray(base_adj.reshape(IC, 128).T)  # [p, ic]
    _WEIGHT_CACHE[key] = (g8, base_tile)
    return g8, base_tile


def kernel(x: np.ndarray, coeffs: np.ndarray) -> np.ndarray:
    assert x.shape == (8, 2048, IN_F) and coeffs.shape == (OUT_F, IN_F, 12)
    t = np.linspace(0.0, 1.0, 10, dtype=np.float32)  # exact knots of reference

    # Segment index per element via the same float32 comparisons the
    # reference uses (bit-exact segment assignment); 0..8 exact in bf16.
    xf = np.ascontiguousarray(x.reshape(-1, IN_F))  # [16384, 512]
    seg = np.zeros(xf.shape, dtype=np.float32)
    for m in range(1, 9):
        seg += (xf >= t[m]).astype(np.float32)
    segb_dev = np.ascontiguousarray(
        seg.T.reshape(4, 128, N_CORES * TOK).transpose(1, 0, 2)
    ).astype(ml_dtypes.bfloat16)  # [p, jb, T]

    g8, base_tile = _prep_weights(coeffs)

    in_maps = []
    for core in range(N_CORES):
        sl = slice(core * TOK, (core + 1) * TOK)
        in_maps.append(
            {
                "segb": np.ascontiguousarray(segb_dev[:, :, sl]),
                "g8": g8,
                "base": base_tile,
            }
        )

    nc = _get_program()
    res = run_bass_kernel_spmd(nc, in_maps, core_ids=list(range(N_CORES)))
    # out[ic, p, tok] (bf16) -> [tok, i] f32
    out = np.stack(
        [
            res.results[core]["out"].reshape(OUT_F, TOK).T.astype(np.float32)
            for core in range(N_CORES)
        ]
    )
    return np.ascontiguousarray(out)


# revision 23
# speedup vs baseline: 1.0094x; 1.0094x over previous
"""Trainium2 Bass kernel for nn_KANLayer (piecewise-constant KAN forward).

Math: out[b,t,i] = sum_j D[i,j,seg(x_tj)] with D[i,j,s] = c_s+c_{s+1}+c_{s+2},
seg = which of the 9 knot intervals x falls in. Telescoping over s with step
planes step_s[t,j] = [seg >= s] gives a K=512*8=4096 matmul with a 0/1 left
operand plus a free per-i constant (base).

Full-fp8 design (vs the 127.9us bf16 baseline, PE-bound at 109us):
  * All 4096 K-lanes run in fp8e4m3 with DoubleRow perf mode (2 K-lanes per
    PE cell per cycle; measured on HW: DR matmuls pace at the same 216ns as
    bf16 N=512). PE work: 256 MMs x 216ns ~= 55us.
  * The G_s increments are quantized by a per-lane beam-search DP minimizing
    the seg-uniform expected squared error of the cumulative sums, with the
    per-lane mean error folded into the exact-f32 base (mean-centering).
    Measured end-to-end (deterministic fixed-seed inputs): rel err 1.6e-2.
  * g-stationary orientation: out[i-chunk, token]; LDWEIGHTS (135ns) hides
    under the 216ns matmuls.
  * Step-plane builds are split across two engines so neither is critical:
    chunks 0-7 (j 0..255) as 0/1 is_ge planes on VectorE, chunks 8-15
    (j 256..511) as +-1 sign planes on ScalarE with halved coefficients
    (exact exponent shift -> identical precision). Build FIFOs are ordered
    so evacuations never sit behind not-yet-consumed builds.
  * PSUM evacuation adds base and downcasts to bf16 (half the out DMA and
    an accelerated DVE/ACT mode); the host upcasts. Out DMAs ride the
    otherwise-idle sync ring.

Sharding: data-parallel, 2048 tokens per core; g/base replicated. Output is
produced transposed ([i, token]) and untransposed on the host.
"""

import hashlib
from contextlib import ExitStack

import numpy as np
import ml_dtypes

import concourse.bass as bass  # noqa: F401
import concourse.tile as tile
from concourse import bacc, mybir
from concourse.bass_utils import run_bass_kernel_spmd

N_CORES = 8
TOK = 2048          # tokens per core
IN_F = 512
OUT_F = 512
GTOK = 512          # tokens per group
N_GRP = TOK // GTOK  # 4
IC = 4              # i-feature chunks of 128
NC8 = 16            # fp8 DoubleRow chunks: (j-block of 128) x (s-pair), 4x4
# chunks built on ScalarE as sign (+-1) planes with halved coefficients; the
# rest are is_ge (0/1) planes on VectorE. Alternating over the early chunks
# makes the combined build pace (~1.1us/chunk) beat the PE's consumption
# (1.73us/chunk) from a standing start; vector ~455ns/op vs scalar ~700ns/op
# puts 10 chunks on vector and 6 on scalar.
SIGN_CHUNKS = (1, 3, 5, 7, 9, 11)
BF16 = mybir.dt.bfloat16
F8 = mybir.dt.float8e4
F32 = mybir.dt.float32
E4NP = ml_dtypes.float8_e4m3

_PROGRAM_CACHE = {}
_WEIGHT_CACHE = {}


def _build_program():
    nc = bacc.Bacc("TRN2", target_bir_lowering=False, debug=False)

    segb_d = nc.dram_tensor("segb", [128, 4, TOK], BF16, kind="ExternalInput").ap()
    g8_d = nc.dram_tensor("g8", [128, NC8, 2, IC, 128], F8, kind="ExternalInput").ap()
    base_d = nc.dram_tensor("base", [128, IC], F32, kind="ExternalInput").ap()
    # out[ic, p, tok] -> feature i = ic*128 + p (bf16; host upcasts)
    out_d = nc.dram_tensor("out", [IC, 128, TOK], BF16, kind="ExternalOutput").ap()

    with tile.TileContext(nc) as tc, ExitStack() as ctx:
        seg_pool = ctx.enter_context(tc.tile_pool(name="seg", bufs=1))
        g8_pool = ctx.enter_context(tc.tile_pool(name="g8", bufs=1))
        base_pool = ctx.enter_context(tc.tile_pool(name="base", bufs=1))
        wm_pool = ctx.enter_context(tc.tile_pool(name="wm", bufs=1))
        st8_pool = ctx.enter_context(tc.tile_pool(name="st8", bufs=1))
        out_pool = ctx.enter_context(tc.tile_pool(name="out", bufs=4))
        psum_pool = ctx.enter_context(tc.tile_pool(name="psum", bufs=1, space="PSUM"))

        # --- PE warmup: un-throttle the HAM clock gate before real work
        # (covers the ~3.4us activity window plus the input-DMA latency).
        wm = wm_pool.tile([128, 384], BF16, name="wm")
        nc.vector.memset(wm[:], 0.0)
        ps_w = psum_pool.tile([128, 2 * GTOK], F32, name="ps_0")
        for _ in range(23):
            nc.tensor.matmul(
                ps_w[:, :256], wm[:, :128], wm[:, 128:384],
                start=True, stop=True, skip_group_check=True,
            )

        # --- input DMAs.  seg pieces on the sync HWDGE ring (group-major so
        # group 0 lands first), g chunks on the scalar ring in consumption
        # order (first 4 singly for a fast start, the rest batched to keep
        # the scalar engine's issue cost low), base on gpsimd.
        # seg pieces jb-major per super-half: the matmul stream consumes
        # chunks jb-major, and each piece covers BOTH groups of a super, so
        # piece k is needed ~7us after piece k-1 while they land ~1.4us
        # apart. (Group-major 16-piece order starved the early builds: the
        # piece for group 1 / jb0 -- needed by the second matmul -- was 5th.)
        segb_t = seg_pool.tile([128, 4, TOK], BF16, name="segb")
        for half in range(2):
            sl = slice(half * 2 * GTOK, (half + 1) * 2 * GTOK)
            for jb in range(4):
                nc.sync.dma_start(segb_t[:, jb, sl], segb_d[:, jb, sl])

        g8_t = g8_pool.tile([128, NC8, 2, IC, 128], F8, name="g8")
        for c0, c1 in ((0, 4), (4, 16)):
            nc.scalar.dma_start(g8_t[:, c0:c1], g8_d[:, c0:c1])

        base_t = base_pool.tile([128, IC], F32, name="base")
        nc.gpsimd.dma_start(base_t[:], base_d[:])

        # per-level sign biases (0.5 - s) as [128,1] columns for ACT sign
        bias_t = base_pool.tile([128, 8], F32, name="sgnbias")
        for s in range(1, 9):
            nc.gpsimd.memset(bias_t[:, s - 1 : s], 0.5 - float(s))

        # --- step-plane builds. chunk c = jb*4 + sp covers lanes
        # (j = jb*128 + p, s = 2*sp + 1 + b). VectorE owns chunks 0..7
        # (is_ge -> 0/1), ScalarE owns chunks 8..15 (sign -> -1/+1, with
        # halved g so the net step coefficient is unchanged). Interleaved by
        # chunk across the two groups of each super so the PE never waits.
        st8 = [st8_pool.tile([128, NC8, 2, GTOK], F8, name=f"st8_{q}") for q in range(N_GRP)]

        def emit_build(c, q):
            jb, sp = c // 4, c % 4
            sl = slice(q * GTOK, (q + 1) * GTOK)
            for b in range(2):
                s = 2 * sp + 1 + b
                if c in SIGN_CHUNKS:
                    nc.scalar.sign(
                        st8[q][:, c, b, :], segb_t[:, jb, sl],
                        bias=bias_t[:, s - 1 : s],
                    )
                else:
                    nc.vector.tensor_scalar(
                        st8[q][:, c, b, :], segb_t[:, jb, sl],
                        float(s) - 0.5, None, mybir.AluOpType.is_ge,
                    )

        def emit_mms(sup, ps):
            groups = (2 * sup, 2 * sup + 1)
            for c in range(NC8):
                for ic in range(IC):
                    for qi, q in enumerate(groups):
                        nc.tensor.matmul(
                            ps[ic][:, qi * GTOK : (qi + 1) * GTOK],
                            g8_t[:, c, :, ic, :],
                            st8[q][:, c, :, :],
                            start=c == 0, stop=c == NC8 - 1,
                            perf_mode=mybir.MatmulPerfMode.DoubleRow,
                        )

        def emit_evac(sup, ps, ic, on_vector, ndma, split=False):
            # one [128, 1024] op evacuates the whole i-chunk (both banks),
            # adds base, downcasts to bf16; one contiguous DMA ships it.
            # split=True (final super) halves it across VectorE+ScalarE so
            # the post-last-matmul tail shrinks.
            groups = (2 * sup, 2 * sup + 1)
            ot = out_pool.tile([128, 2 * GTOK], BF16, name="ot")
            if split:
                nc.vector.tensor_scalar(
                    ot[:, :GTOK], ps[ic][:, :GTOK], base_t[:, ic : ic + 1],
                    None, mybir.AluOpType.add,
                )
                nc.scalar.add(
                    ot[:, GTOK:], ps[ic][:, GTOK:], base_t[:, ic : ic + 1]
                )
            elif on_vector:
                nc.vector.tensor_scalar(
                    ot[:], ps[ic][:], base_t[:, ic : ic + 1],
                    None, mybir.AluOpType.add,
                )
            else:
                nc.scalar.add(ot[:], ps[ic][:], base_t[:, ic : ic + 1])
            eng = nc.sync if ndma % 2 == 0 else nc.gpsimd
            eng.dma_start(
                out_d[ic][:, groups[0] * GTOK : (groups[0] + 2) * GTOK], ot[:]
            )

        # Emission order sets each engine's FIFO. Builds are emitted as
        # early as possible; each engine slots its share of the super-0
        # evacuations (gated by super-0's stop matmuls) between super-1
        # build segments whose deadlines leave room, so neither delays the
        # other. Evacuations go ic-interleaved V/S in bank-reuse order.
        ps0 = [psum_pool.tile([128, 2 * GTOK], F32, name=f"ps_{ic}")
               for ic in range(IC)]
        for c in range(NC8):
            for q in (0, 1):
                emit_build(c, q)
        # super-1 builds, early part: vector chunks 0-4, scalar chunks 1,3
        for c in (0, 1, 2, 3, 4):
            for q in (2, 3):
                emit_build(c, q)
        emit_mms(0, ps0)
        emit_evac(0, ps0, 0, False, 0)
        emit_evac(0, ps0, 1, True, 1)
        emit_evac(0, ps0, 2, False, 0)
        emit_evac(0, ps0, 3, True, 1)
        ps1 = [psum_pool.tile([128, 2 * GTOK], F32, name=f"ps_{ic}")
               for ic in range(IC)]
        # super-1 builds, late part
        for c in (5, 6, 7, 8, 9, 10, 11, 12, 13, 14, 15):
            for q in (2, 3):
                emit_build(c, q)
        emit_mms(1, ps1)
        for ic in range(IC):
            emit_evac(1, ps1, ic, False, ic % 2, split=True)

    nc.compile()
    return nc


def _get_program():
    if "nc" not in _PROGRAM_CACHE:
        _PROGRAM_CACHE["nc"] = _build_program()
    return _PROGRAM_CACHE["nc"]


# sorted finite fp8e4m3 grid for the DP quantizer
_E4_GRID = np.arange(256, dtype=np.uint8).view(E4NP).astype(np.float32)
_E4_GRID = np.unique(_E4_GRID[np.isfinite(_E4_GRID)])


def _dp_quant(Gsub: np.ndarray, B: int = 8) -> np.ndarray:
    """Beam-DP quantization of cumulative increments onto the fp8e4m3 grid.

    Minimizes sum_s e_s^2 - (sum_s e_s)^2/9 per lane, where e_s is the
    partial-sum error at level s (seg uniform over 9 values; the mean term
    is folded into base by the caller)."""
    grid, NG = _E4_GRID, len(_E4_GRID)
    S, L = Gsub.shape
    P = np.cumsum(Gsub, axis=0)
    Pq = np.zeros((1, L), np.float32)
    se = np.zeros((1, L), np.float32)
    s2 = np.zeros((1, L), np.float32)
    paths = np.zeros((1, S, L), np.float32)
    for s in range(S):
        tgt = P[s][None, :] - Pq
        idx = np.searchsorted(grid, tgt.ravel()).reshape(tgt.shape)
        B0 = Pq.shape[0]
        offs = (-1, 0)
        cand = [grid[np.clip(idx + o, 0, NG - 1)] for o in offs]
        newPq = np.concatenate([Pq + qc for qc in cand], axis=0)
        e = P[s][None, :] - newPq
        newse = np.tile(se, (len(offs), 1)) + e
        news2 = np.tile(s2, (len(offs), 1)) + e * e
        newpaths = np.tile(paths, (len(offs), 1, 1))
        for k, qc in enumerate(cand):
            newpaths[k * B0 : (k + 1) * B0, s, :] = qc
        rem = S - 1 - s
        pse = newse + e * rem
        ps2 = news2 + e * e * rem
        cost = ps2 - pse * pse / 9.0
        B_eff = min(B, cost.shape[0])
        sel = np.argpartition(cost, B_eff - 1, axis=0)[:B_eff]
        Pq = np.take_along_axis(newPq, sel, 0)
        se = np.take_along_axis(newse, sel, 0)
        s2 = np.take_along_axis(news2, sel, 0)
        paths = np.take_along_axis(newpaths, sel[:, None, :], 0)
    best = np.argmin(s2 - se * se / 9.0, axis=0)
    return np.take_along_axis(paths, best[None, None, :], 0)[0]


def _prep_weights(coeffs: np.ndarray):
    key = hashlib.sha1(np.ascontiguousarray(coeffs).tobytes()).hexdigest()
    if key in _WEIGHT_CACHE:
        return _WEIGHT_CACHE[key]
    c = coeffs.astype(np.float32)
    # G[s-1][j, i] = c[i,j,s+2] - c[i,j,s-1]; base[i] = sum_j (c0+c1+c2)
    G = np.empty((8, IN_F, OUT_F), dtype=np.float32)
    for s in range(1, 9):
        G[s - 1] = (c[:, :, s + 2] - c[:, :, s - 1]).T
    base = (c[:, :, 0] + c[:, :, 1] + c[:, :, 2]).sum(axis=1).astype(np.float32)

    Gq = _dp_quant(G.reshape(8, -1)).reshape(8, IN_F, OUT_F)
    # sign-plane chunks (SIGN_CHUNKS, built as +-1 on ScalarE): store h = Gq/2
    # (the fp8 grid is exponent-self-similar, so this is exact except at the
    # subnormal floor). sum_s h*sgn = sum_s (2h)*step - sum_s h, so the
    # effective step coefficient is 2h and the constant folds into base.
    G_store = Gq.copy()
    Gq_eff = Gq.copy()
    hsum = np.zeros(OUT_F, dtype=np.float32)
    for c8 in SIGN_CHUNKS:
        jb, sp = c8 // 4, c8 % 4
        jsl = slice(jb * 128, (jb + 1) * 128)
        for s_idx in (2 * sp, 2 * sp + 1):
            h = (Gq[s_idx, jsl, :] * 0.5).astype(E4NP).astype(np.float32)
            G_store[s_idx, jsl, :] = h
            Gq_eff[s_idx, jsl, :] = 2.0 * h
            hsum += h.sum(axis=0)
    # mean-centering with the effective coefficients + sign-plane constant
    E = np.cumsum(G, axis=0) - np.cumsum(Gq_eff, axis=0)
    base_adj = base + (E.sum(axis=0) / 9.0).sum(axis=0) + hsum

    # g8[p, c8 = jb*4+sp, b, ic, m] = G_store[2*sp+b, jb*128+p, ic*128+m]
    Gf = G_store.reshape(4, 2, 4, 128, IC, 128)  # [sp, b, jb, p, ic, m]
    g8 = np.ascontiguousarray(
        Gf.transpose(3, 2, 0, 1, 4, 5).reshape(128, NC8, 2, IC, 128)
    ).astype(E4NP)
    base_tile = np.ascontiguousarray(base_adj.reshape(IC, 128).T)  # [p, ic]
    _WEIGHT_CACHE[key] = (g8, base_tile)
    return g8, base_tile


def kernel(x: np.ndarray, coeffs: np.ndarray) -> np.ndarray:
    assert x.shape == (8, 2048, IN_F) and coeffs.shape == (OUT_F, IN_F, 12)
    t = np.linspace(0.0, 1.0, 10, dtype=np.float32)  # exact knots of reference

    # Segment index per element via the same float32 comparisons the
    # reference uses (bit-exact segment assignment); 0..8 exact in bf16.
    xf = np.ascontiguousarray(x.reshape(-1, IN_F))  # [16384, 512]
    seg = np.zeros(xf.shape, dtype=np.float32)
    for m in range(1, 9):
        seg += (xf >= t[m]).astype(np.float32)
    segb_dev = np.ascontiguousarray(
        seg.T.reshape(4, 128, N_CORES * TOK).transpose(1, 0, 2)
    ).astype(ml_dtypes.bfloat16)  # [p, jb, T]

    g8, base_tile = _prep_weights(coeffs)

    in_maps = []
    for core in range(N_CORES):
        sl = slice(core * TOK, (core + 1) * TOK)
        in_maps.append(
            {
                "segb": np.ascontiguousarray(segb_dev[:, :, sl]),
                "g8": g8,
                "base": base_tile,
            }
        )

    nc = _get_program()
    res = run_bass_kernel_spmd(nc, in_maps, core_ids=list(range(N_CORES)))
    # out[ic, p, tok] (bf16) -> [tok, i] f32
    out = np.stack(
        [
            res.results[core]["out"].reshape(OUT_F, TOK).T.astype(np.float32)
            for core in range(N_CORES)
        ]
    )
    return np.ascontiguousarray(out)


# revision 25
# speedup vs baseline: 1.0168x; 1.0074x over previous
"""Trainium2 Bass kernel for nn_KANLayer (piecewise-constant KAN forward).

Math: out[b,t,i] = sum_j D[i,j,seg(x_tj)] with D[i,j,s] = c_s+c_{s+1}+c_{s+2},
seg = which of the 9 knot intervals x falls in. Telescoping over s with step
planes step_s[t,j] = [seg >= s] gives a K=512*8=4096 matmul with a 0/1 left
operand plus a free per-i constant (base).

Full-fp8 design (vs the 127.9us bf16 baseline, PE-bound at 109us):
  * All 4096 K-lanes run in fp8e4m3 with DoubleRow perf mode (2 K-lanes per
    PE cell per cycle; measured on HW: DR matmuls pace at the same 216ns as
    bf16 N=512). PE work: 256 MMs x 216ns ~= 55us.
  * The G_s increments are quantized by a per-lane beam-search DP minimizing
    the seg-uniform expected squared error of the cumulative sums, with the
    per-lane mean error folded into the exact-f32 base (mean-centering).
    Measured end-to-end (deterministic fixed-seed inputs): rel err 1.6e-2.
  * g-stationary orientation: out[i-chunk, token]; LDWEIGHTS (135ns) hides
    under the 216ns matmuls.
  * Step-plane builds are split across two engines so neither is critical:
    SIGN_CHUNKS as +-1 sign planes on ScalarE (with halved coefficients --
    an exact exponent shift, so identical precision), the rest as 0/1
    is_ge planes on VectorE. Each engine's FIFO is ordered so builds stay
    ahead of the matmul stream and evacuations never block builds.
  * PSUM evacuation adds base and downcasts to bf16 (half the out DMA and
    an accelerated DVE/ACT mode); the host upcasts. Out DMAs ride the
    otherwise-idle sync ring.

Sharding: data-parallel, 2048 tokens per core; g/base replicated. Output is
produced transposed ([i, token]) and untransposed on the host.
"""

import hashlib
from contextlib import ExitStack

import numpy as np
import ml_dtypes

import concourse.bass as bass  # noqa: F401
import concourse.tile as tile
from concourse import bacc, mybir
from concourse.bass_utils import run_bass_kernel_spmd

N_CORES = 8
TOK = 2048          # tokens per core
IN_F = 512
OUT_F = 512
GTOK = 512          # tokens per group
N_GRP = TOK // GTOK  # 4
IC = 4              # i-feature chunks of 128
NC8 = 16            # fp8 DoubleRow chunks: (j-block of 128) x (s-pair), 4x4
# chunks built on ScalarE as sign (+-1) planes with halved coefficients; the
# rest are is_ge (0/1) planes on VectorE. Alternating over the early chunks
# makes the combined build pace (~1.1us/chunk) beat the PE's consumption
# (1.73us/chunk) from a standing start; vector ~455ns/op vs scalar ~700ns/op
# puts 10 chunks on vector and 6 on scalar.
SIGN_CHUNKS = (1, 3, 5, 7, 9, 11)
BF16 = mybir.dt.bfloat16
F8 = mybir.dt.float8e4
F32 = mybir.dt.float32
E4NP = ml_dtypes.float8_e4m3

_PROGRAM_CACHE = {}
_WEIGHT_CACHE = {}


def _build_program():
    nc = bacc.Bacc("TRN2", target_bir_lowering=False, debug=False)

    segb_d = nc.dram_tensor("segb", [128, 4, TOK], BF16, kind="ExternalInput").ap()
    g8_d = nc.dram_tensor("g8", [128, NC8, 2, IC, 128], F8, kind="ExternalInput").ap()
    base_d = nc.dram_tensor("base", [128, IC], F32, kind="ExternalInput").ap()
    # out[ic, p, tok] -> feature i = ic*128 + p (bf16; host upcasts)
    out_d = nc.dram_tensor("out", [IC, 128, TOK], BF16, kind="ExternalOutput").ap()

    with tile.TileContext(nc) as tc, ExitStack() as ctx:
        seg_pool = ctx.enter_context(tc.tile_pool(name="seg", bufs=1))
        g8_pool = ctx.enter_context(tc.tile_pool(name="g8", bufs=1))
        base_pool = ctx.enter_context(tc.tile_pool(name="base", bufs=1))
        wm_pool = ctx.enter_context(tc.tile_pool(name="wm", bufs=1))
        st8_pool = ctx.enter_context(tc.tile_pool(name="st8", bufs=1))
        out_pool = ctx.enter_context(tc.tile_pool(name="out", bufs=4))
        psum_pool = ctx.enter_context(tc.tile_pool(name="psum", bufs=1, space="PSUM"))

        # --- PE warmup: un-throttle the HAM clock gate before real work
        # (covers the ~3.4us activity window plus the input-DMA latency).
        wm = wm_pool.tile([128, 384], BF16, name="wm")
        nc.vector.memset(wm[:], 0.0)
        ps_w = psum_pool.tile([128, 2 * GTOK], F32, name="ps_0")
        for _ in range(22):
            nc.tensor.matmul(
                ps_w[:, :256], wm[:, :128], wm[:, 128:384],
                start=True, stop=True, skip_group_check=True,
            )

        # --- input DMAs.  seg on the sync HWDGE ring, g8 on the scalar
        # ring in consumption order, base on gpsimd.
        # seg pieces jb-major per super-half: the matmul stream consumes
        # chunks jb-major, and each piece covers BOTH groups of a super, so
        # piece k is needed ~7us after piece k-1 while they land ~1.4us
        # apart. (Group-major 16-piece order starved the early builds: the
        # piece for group 1 / jb0 -- needed by the second matmul -- was 5th.)
        segb_t = seg_pool.tile([128, 4, TOK], BF16, name="segb")
        for half in range(2):
            sl = slice(half * 2 * GTOK, (half + 1) * 2 * GTOK)
            for jb in range(4):
                nc.sync.dma_start(segb_t[:, jb, sl], segb_d[:, jb, sl])

        g8_t = g8_pool.tile([128, NC8, 2, IC, 128], F8, name="g8")
        for c0, c1 in ((0, 4), (4, 16)):
            nc.scalar.dma_start(g8_t[:, c0:c1], g8_d[:, c0:c1])

        base_t = base_pool.tile([128, IC], F32, name="base")
        nc.gpsimd.dma_start(base_t[:], base_d[:])

        # per-level sign biases (0.5 - s) as [128,1] columns for ACT sign
        bias_t = base_pool.tile([128, 8], F32, name="sgnbias")
        for s in range(1, 9):
            nc.gpsimd.memset(bias_t[:, s - 1 : s], 0.5 - float(s))

        # --- step-plane builds. chunk c = jb*4 + sp covers lanes
        # (j = jb*128 + p, s = 2*sp + 1 + b). VectorE builds is_ge (0/1)
        # planes, ScalarE builds sign (-1/+1) planes whose g is halved so
        # the net step coefficient is unchanged.
        st8 = [st8_pool.tile([128, NC8, 2, GTOK], F8, name=f"st8_{q}") for q in range(N_GRP)]

        def emit_build(c, q):
            jb, sp = c // 4, c % 4
            sl = slice(q * GTOK, (q + 1) * GTOK)
            for b in range(2):
                s = 2 * sp + 1 + b
                if c in SIGN_CHUNKS:
                    nc.scalar.sign(
                        st8[q][:, c, b, :], segb_t[:, jb, sl],
                        bias=bias_t[:, s - 1 : s],
                    )
                else:
                    nc.vector.tensor_scalar(
                        st8[q][:, c, b, :], segb_t[:, jb, sl],
                        float(s) - 0.5, None, mybir.AluOpType.is_ge,
                    )

        def emit_mms(sup, ps):
            groups = (2 * sup, 2 * sup + 1)
            for c in range(NC8):
                for ic in range(IC):
                    for qi, q in enumerate(groups):
                        nc.tensor.matmul(
                            ps[ic][:, qi * GTOK : (qi + 1) * GTOK],
                            g8_t[:, c, :, ic, :],
                            st8[q][:, c, :, :],
                            start=c == 0, stop=c == NC8 - 1,
                            perf_mode=mybir.MatmulPerfMode.DoubleRow,
                        )

        def emit_evac(sup, ps, ic, on_vector, ndma, split=False):
            # one [128, 1024] op evacuates the whole i-chunk (both banks),
            # adds base, downcasts to bf16; one contiguous DMA ships it.
            # split=True (final super) halves it across VectorE+ScalarE so
            # the post-last-matmul tail shrinks.
            groups = (2 * sup, 2 * sup + 1)
            ot = out_pool.tile([128, 2 * GTOK], BF16, name="ot")
            if split:
                nc.vector.tensor_scalar(
                    ot[:, :GTOK], ps[ic][:, :GTOK], base_t[:, ic : ic + 1],
                    None, mybir.AluOpType.add,
                )
                nc.scalar.add(
                    ot[:, GTOK:], ps[ic][:, GTOK:], base_t[:, ic : ic + 1]
                )
            elif on_vector:
                nc.vector.tensor_scalar(
                    ot[:], ps[ic][:], base_t[:, ic : ic + 1],
                    None, mybir.AluOpType.add,
                )
            else:
                nc.scalar.add(ot[:], ps[ic][:], base_t[:, ic : ic + 1])
            eng = nc.sync if ndma % 2 == 0 else nc.gpsimd
            eng.dma_start(
                out_d[ic][:, groups[0] * GTOK : (groups[0] + 2) * GTOK], ot[:]
            )

        # Emission order sets each engine's FIFO. Builds are emitted as
        # early as possible; each engine slots its share of the super-0
        # evacuations (gated by super-0's stop matmuls) between super-1
        # build segments whose deadlines leave room, so neither delays the
        # other. Evacuations go ic-interleaved V/S in bank-reuse order.
        ps0 = [psum_pool.tile([128, 2 * GTOK], F32, name=f"ps_{ic}")
               for ic in range(IC)]
        for c in range(NC8):
            for q in (0, 1):
                emit_build(c, q)
        # super-1 builds, early part: vector chunks 0-4, scalar chunks 1,3
        for c in (0, 1, 2, 3, 4):
            for q in (2, 3):
                emit_build(c, q)
        emit_mms(0, ps0)
        emit_evac(0, ps0, 0, False, 0)
        emit_evac(0, ps0, 1, True, 1)
        emit_evac(0, ps0, 2, False, 0)
        emit_evac(0, ps0, 3, True, 1)
        ps1 = [psum_pool.tile([128, 2 * GTOK], F32, name=f"ps_{ic}")
               for ic in range(IC)]
        # super-1 builds, late part
        for c in (5, 6, 7, 8, 9, 10, 11, 12, 13, 14, 15):
            for q in (2, 3):
                emit_build(c, q)
        emit_mms(1, ps1)
        emit_evac(1, ps1, 0, False, 0)
        emit_evac(1, ps1, 1, True, 1)
        emit_evac(1, ps1, 2, False, 0)
        emit_evac(1, ps1, 3, True, 1)

    nc.compile()
    return nc


def _get_program():
    if "nc" not in _PROGRAM_CACHE:
        _PROGRAM_CACHE["nc"] = _build_program()
    return _PROGRAM_CACHE["nc"]


# sorted finite fp8e4m3 grid for the DP quantizer
_E4_GRID = np.arange(256, dtype=np.uint8).view(E4NP).astype(np.float32)
_E4_GRID = np.unique(_E4_GRID[np.isfinite(_E4_GRID)])


def _dp_quant(Gsub: np.ndarray, B: int = 8) -> np.ndarray:
    """Beam-DP quantization of cumulative increments onto the fp8e4m3 grid.

    Minimizes sum_s e_s^2 - (sum_s e_s)^2/9 per lane, where e_s is the
    partial-sum error at level s (seg uniform over 9 values; the mean term
    is folded into base by the caller)."""
    grid, NG = _E4_GRID, len(_E4_GRID)
    S, L = Gsub.shape
    P = np.cumsum(Gsub, axis=0)
    Pq = np.zeros((1, L), np.float32)
    se = np.zeros((1, L), np.float32)
    s2 = np.zeros((1, L), np.float32)
    paths = np.zeros((1, S, L), np.float32)
    for s in range(S):
        tgt = P[s][None, :] - Pq
        idx = np.searchsorted(grid, tgt.ravel()).reshape(tgt.shape)
        B0 = Pq.shape[0]
        offs = (-1, 0)
        cand = [grid[np.clip(idx + o, 0, NG - 1)] for o in offs]
        newPq = np.concatenate([Pq + qc for qc in cand], axis=0)
        e = P[s][None, :] - newPq
        newse = np.tile(se, (len(offs), 1)) + e
        news2 = np.tile(s2, (len(offs), 1)) + e * e
        newpaths = np.tile(paths, (len(offs), 1, 1))
        for k, qc in enumerate(cand):
            newpaths[k * B0 : (k + 1) * B0, s, :] = qc
        rem = S - 1 - s
        pse = newse + e * rem
        ps2 = news2 + e * e * rem
        cost = ps2 - pse * pse / 9.0
        B_eff = min(B, cost.shape[0])
        sel = np.argpartition(cost, B_eff - 1, axis=0)[:B_eff]
        Pq = np.take_along_axis(newPq, sel, 0)
        se = np.take_along_axis(newse, sel, 0)
        s2 = np.take_along_axis(news2, sel, 0)
        paths = np.take_along_axis(newpaths, sel[:, None, :], 0)
    best = np.argmin(s2 - se * se / 9.0, axis=0)
    return np.take_along_axis(paths, best[None, None, :], 0)[0]


def _prep_weights(coeffs: np.ndarray):
    key = hashlib.sha1(np.ascontiguousarray(coeffs).tobytes()).hexdigest()
    if key in _WEIGHT_CACHE:
        return _WEIGHT_CACHE[key]
    c = coeffs.astype(np.float32)
    # G[s-1][j, i] = c[i,j,s+2] - c[i,j,s-1]; base[i] = sum_j (c0+c1+c2)
    G = np.empty((8, IN_F, OUT_F), dtype=np.float32)
    for s in range(1, 9):
        G[s - 1] = (c[:, :, s + 2] - c[:, :, s - 1]).T
    base = (c[:, :, 0] + c[:, :, 1] + c[:, :, 2]).sum(axis=1).astype(np.float32)

    Gq = _dp_quant(G.reshape(8, -1)).reshape(8, IN_F, OUT_F)
    # sign-plane chunks (SIGN_CHUNKS, built as +-1 on ScalarE): store h = Gq/2
    # (the fp8 grid is exponent-self-similar, so this is exact except at the
    # subnormal floor). sum_s h*sgn = sum_s (2h)*step - sum_s h, so the
    # effective step coefficient is 2h and the constant folds into base.
    G_store = Gq.copy()
    Gq_eff = Gq.copy()
    hsum = np.zeros(OUT_F, dtype=np.float32)
    for c8 in SIGN_CHUNKS:
        jb, sp = c8 // 4, c8 % 4
        jsl = slice(jb * 128, (jb + 1) * 128)
        for s_idx in (2 * sp, 2 * sp + 1):
            h = (Gq[s_idx, jsl, :] * 0.5).astype(E4NP).astype(np.float32)
            G_store[s_idx, jsl, :] = h
            Gq_eff[s_idx, jsl, :] = 2.0 * h
            hsum += h.sum(axis=0)
    # mean-centering with the effective coefficients + sign-plane constant
    E = np.cumsum(G, axis=0) - np.cumsum(Gq_eff, axis=0)
    base_adj = base + (E.sum(axis=0) / 9.0).sum(axis=0) + hsum

    # g8[p, c8 = jb*4+sp, b, ic, m] = G_store[2*sp+b, jb*128+p, ic*128+m]
    Gf = G_store.reshape(4, 2, 4, 128, IC, 128)  # [sp, b, jb, p, ic, m]
    g8 = np.ascontiguousarray(
        Gf.transpose(3, 2, 0, 1, 4, 5).reshape(128, NC8, 2, IC, 128)
    ).astype(E4NP)
    base_tile = np.ascontiguousarray(base_adj.reshape(IC, 128).T)  # [p, ic]
    _WEIGHT_CACHE[key] = (g8, base_tile)
    return g8, base_tile


def kernel(x: np.ndarray, coeffs: np.ndarray) -> np.ndarray:
    assert x.shape == (8, 2048, IN_F) and coeffs.shape == (OUT_F, IN_F, 12)
    t = np.linspace(0.0, 1.0, 10, dtype=np.float32)  # exact knots of reference

    # Segment index per element via the same float32 comparisons the
    # reference uses (bit-exact segment assignment); 0..8 exact in bf16.
    xf = np.ascontiguousarray(x.reshape(-1, IN_F))  # [16384, 512]
    seg = np.zeros(xf.shape, dtype=np.float32)
    for m in range(1, 9):
        seg += (xf >= t[m]).astype(np.float32)
    segb_dev = np.ascontiguousarray(
        seg.T.reshape(4, 128, N_CORES * TOK).transpose(1, 0, 2)
    ).astype(ml_dtypes.bfloat16)  # [p, jb, T]

    g8, base_tile = _prep_weights(coeffs)

    in_maps = []
    for core in range(N_CORES):
        sl = slice(core * TOK, (core + 1) * TOK)
        in_maps.append(
            {
                "segb": np.ascontiguousarray(segb_dev[:, :, sl]),
                "g8": g8,
                "base": base_tile,
            }
        )

    nc = _get_program()
    res = run_bass_kernel_spmd(nc, in_maps, core_ids=list(range(N_CORES)))
    # out[ic, p, tok] (bf16) -> [tok, i] f32
    out = np.stack(
        [
            res.results[core]["out"].reshape(OUT_F, TOK).T.astype(np.float32)
            for core in range(N_CORES)
        ]
    )
    return np.ascontiguousarray(out)
